# revision 1
# baseline (speedup 1.0000x reference)
"""BiMambaBlock Trainium2 kernel.

Full inputs in, full output out. Internally: 8-way SPMD shard over
(batch=2) x (direction fwd/bwd) x (d_inner half). Each core runs the same
Bass/Tile program on its own data:

  LN (stats in time-major) -> PE transpose -> in_proj (x full 1536, z for its
  768-half) -> depthwise causal conv as 4 accumulating diagonal matmuls ->
  SiLU -> x_proj (interleaved psum accumulation) -> dt_proj + softplus
  (exp+ln) -> per-(n, d-tile) SSM scan on the DVE scan instruction ->
  y = sum_n C_n*h_n + D*x -> gate with silu(z) -> out_proj partial.

The bwd direction is the same program on a time-reversed copy of u; its
partial output is reversed back on the host. The 4 partials per batch are
summed on the host (row-parallel out_proj) and the residual is added.
"""

import os
import numpy as np

import concourse.bass as bass
import concourse.bacc as bacc
import concourse.mybir as mybir
import concourse.tile as tile
from concourse.bass_utils import run_bass_kernel_spmd

F32 = mybir.dt.float32
F32R = mybir.dt.float32r
AF = mybir.ActivationFunctionType
OP = mybir.AluOpType

D_MODEL = 768
D_INNER = 1536
N_STATE = 16
DT_RANK = 96
K_CONV = 4
B, L = 2, 1024
DSH = D_INNER // 2          # 768 channels scanned per core
NT_M = D_MODEL // 128       # 6 tiles of model dim
NT_DF = D_INNER // 128      # 12 tiles of full d_inner
NT_DS = DSH // 128          # 6 tiles of the scan shard
NT_L = L // 128             # 8 time tiles
NLC = L // 512              # 2 psum column chunks

USE_F32R = os.environ.get("KERNEL_F32", "0") != "1"   # float32r matmuls by default
MMDT = F32R if USE_F32R else F32
BU_DVE_TILES = int(os.environ.get("KERNEL_BU_DVE", "1"))  # Bu d-tiles on DVE
ACC_DVE_TILES = int(os.environ.get("KERNEL_ACC_DVE", "0"))  # y-acc d-tiles on DVE
DBG = os.environ.get("KERNEL_DEBUG", "0") == "1"
BF16_HC = os.environ.get("KERNEL_BF16_HC", "0") == "1"  # bf16 h*C multiply (2x DVE)
BC_BUFS = int(os.environ.get("KERNEL_BC_BUFS", "4"))
TMP_BUFS = int(os.environ.get("KERNEL_TMP_BUFS", "3"))

LAST_RESULTS = None  # BassKernelResults stash for test.py


def _build_program():
    nc = bacc.Bacc("TRN2", target_bir_lowering=False)

    # ---- DRAM I/O (per-core shapes) ----
    u_in = nc.dram_tensor("u_in", [NT_L, 128, D_MODEL], F32, kind="ExternalInput")
    w_inx = nc.dram_tensor("w_inx", [NT_M, 128, D_INNER], MMDT, kind="ExternalInput")
    w_inz = nc.dram_tensor("w_inz", [NT_M, 128, DSH], MMDT, kind="ExternalInput")
    x_bias = nc.dram_tensor("x_bias", [128, NT_DF], F32, kind="ExternalInput")
    z_bias = nc.dram_tensor("z_bias", [128, NT_DS], F32, kind="ExternalInput")
    conv_diag = nc.dram_tensor("conv_diag", [NT_DF, 128, K_CONV, 128], MMDT,
                               kind="ExternalInput")
    conv_b = nc.dram_tensor("conv_b", [128, NT_DF], F32, kind="ExternalInput")
    w_xproj = nc.dram_tensor("w_xproj", [NT_DF, 128, 128], MMDT, kind="ExternalInput")
    w_dt = nc.dram_tensor("w_dt", [DT_RANK, DSH], MMDT, kind="ExternalInput")
    dt_bias = nc.dram_tensor("dt_bias", [128, NT_DS], F32, kind="ExternalInput")
    a_sc = nc.dram_tensor("a_sc", [128, NT_DS, N_STATE], F32, kind="ExternalInput")
    d_vec = nc.dram_tensor("d_vec", [128, NT_DS], F32, kind="ExternalInput")
    w_out = nc.dram_tensor("w_out", [NT_DS, 128, D_MODEL], MMDT, kind="ExternalInput")
    outp = nc.dram_tensor("outp", [NT_M, 128, L], F32, kind="ExternalOutput")
    if DBG:
        dbg_unT = nc.dram_tensor("dbg_unT", [128, L], F32, kind="ExternalOutput")
        dbg_xs = nc.dram_tensor("dbg_xs", [128, L], F32, kind="ExternalOutput")
        dbg_xdbl = nc.dram_tensor("dbg_xdbl", [128, L], F32, kind="ExternalOutput")
        dbg_dt = nc.dram_tensor("dbg_dt", [128, L], F32, kind="ExternalOutput")
        dbg_y = nc.dram_tensor("dbg_y", [128, L], F32, kind="ExternalOutput")


    with tile.TileContext(nc) as tc:
        with (
            tc.tile_pool(name="bigw", bufs=1) as bigw,
            tc.tile_pool(name="wsmall", bufs=1) as wsmall,
            tc.tile_pool(name="wstream", bufs=1) as wstream,
            tc.tile_pool(name="scratch", bufs=1) as scratch,   # u -> conv_in
            tc.tile_pool(name="stat", bufs=4) as stat,
            tc.tile_pool(name="unt", bufs=1) as untp,          # unT -> bu/h
            tc.tile_pool(name="xssm", bufs=1) as xssmp,
            tc.tile_pool(name="zstr", bufs=1) as zstr,
            tc.tile_pool(name="dts", bufs=1) as dtp,
            tc.tile_pool(name="ys", bufs=1) as ysp,
            tc.tile_pool(name="xdbl", bufs=1) as xdblp,
            tc.tile_pool(name="bcp", bufs=1) as bcp,
            tc.tile_pool(name="outsb", bufs=1) as outsb,
            tc.tile_pool(name="ps", bufs=1, space="PSUM") as ps,
            tc.tile_pool(name="drp", bufs=1, space="DRAM") as drp,
        ):
            bc_rows = drp.tile([2 * N_STATE, L], F32, tag="bcr")
            z_dram = drp.tile([NT_DS, 128, L], F32, tag="zdr")
            # identity for PE transposes (fill lands where the compare FAILS)
            ident = wsmall.tile([128, 128], F32, tag="ident")
            nc.vector.memset(ident, 0.0)
            nc.gpsimd.affine_select(
                out=ident, in_=ident, compare_op=OP.not_equal, fill=1.0,
                base=0, pattern=[[-1, 128]], channel_multiplier=1,
            )

            # ---------- Phase 0: LayerNorm (time-major) ----------
            eps = stat.tile([128, 1], F32, tag="eps", bufs=1)
            nc.vector.memset(eps, 1e-5)
            unT = []
            for mt in range(NT_M):
                unt_t = untp.tile([128, L], MMDT, tag="unT", bufs=NT_M,
                                  name=f"unT{mt}")
                unT.append(unt_t)
            for lt in range(NT_L):
                ut = scratch.tile([128, L + K_CONV - 1], F32, tag="scr", bufs=4)
                nc.sync.dma_start(out=ut[:, 0:D_MODEL], in_=u_in[lt])
                sub = ut[:, 0:D_MODEL].rearrange("p (s f) -> p s f", f=256)
                st = stat.tile([128, 3, nc.vector.BN_STATS_DIM], F32, tag="bst")
                for sg in range(3):
                    nc.vector.bn_stats(out=st[:, sg], in_=sub[:, sg])
                mv = stat.tile([128, nc.vector.BN_AGGR_DIM], F32, tag="mv")
                nc.vector.bn_aggr(out=mv, in_=st)
                mean = mv[:, 0:1]
                rstd = stat.tile([128, 1], F32, tag="rstd")
                nc.scalar.activation(out=rstd, in_=mv[:, 1:2], func=AF.Sqrt,
                                     bias=eps[:, 0:1])
                nc.vector.reciprocal(out=rstd, in_=rstd)
                nc.vector.tensor_scalar(out=ut[:, 0:D_MODEL], in0=ut[:, 0:D_MODEL],
                                        scalar1=mean, scalar2=rstd,
                                        op0=OP.subtract, op1=OP.mult)
                # transpose this time tile into all unT column blocks
                pt = ps.tile([128, D_MODEL], F32, tag="big", bufs=2)
                for mt in range(NT_M):
                    nc.tensor.transpose(
                        out=pt[:, mt * 128:(mt + 1) * 128],
                        in_=ut[:, mt * 128:(mt + 1) * 128],
                        identity=ident,
                    )
                for mt in range(NT_M):
                    nc.scalar.activation(
                        out=unT[mt][:, lt * 128:(lt + 1) * 128],
                        in_=pt[:, mt * 128:(mt + 1) * 128], func=AF.Identity)

            if DBG:
                nc.sync.dma_start(out=dbg_unT[:, :], in_=unT[0].bitcast(F32))

            # ---------- Phase 1: in_proj x -> conv -> silu, x_proj interleave ----
            wx = []
            for kt in range(NT_M):
                w = bigw.tile([128, D_INNER], MMDT, tag="big", bufs=NT_M)
                nc.sync.dma_start(out=w, in_=w_inx[kt])
                wx.append(w)
            wxp = []
            for kt in range(NT_DF):
                w = wsmall.tile([128, 128], MMDT, tag=f"wxp{kt}")
                nc.sync.dma_start(out=w, in_=w_xproj[kt])
                wxp.append(w)

            xb_sb = wsmall.tile([128, NT_DF], F32, tag="xb")
            nc.sync.dma_start(out=xb_sb, in_=x_bias[:, :])
            cb_sb = wsmall.tile([128, NT_DF], F32, tag="cb")
            nc.sync.dma_start(out=cb_sb, in_=conv_b[:, :])

            zpad = wsmall.tile([128, K_CONV - 1], F32, tag="zpad")
            nc.vector.memset(zpad, 0.0)

            pt_xp = ps.tile([128, L], F32, tag="xp", bufs=1)
            x_ssm = [None] * NT_DF
            # shard tiles (kept, later overwritten by dtx) go to xssmp;
            # non-shard tiles stream through a small pool.
            for ddt in range(NT_DF):
                cin = scratch.tile([128, L + K_CONV - 1], MMDT, tag="scr", bufs=4)
                nc.vector.tensor_copy(out=cin[:, 0:K_CONV - 1], in_=zpad)
                pt = ps.tile([128, L], F32, tag="big", bufs=2)
                for lc in range(NLC):
                    for kt in range(NT_M):
                        nc.tensor.matmul(
                            out=pt[:, lc * 512:(lc + 1) * 512],
                            lhsT=wx[kt][:, ddt * 128:(ddt + 1) * 128],
                            rhs=unT[kt][:, lc * 512:(lc + 1) * 512],
                            start=(kt == 0), stop=(kt == NT_M - 1),
                        )
                nc.scalar.activation(out=cin[:, K_CONV - 1:], in_=pt,
                                     func=AF.Identity, bias=xb_sb[:, ddt:ddt + 1])
                # conv as 4 accumulating diagonal matmuls + SiLU
                dg = wstream.tile([128, K_CONV, 128], MMDT, tag="diag", bufs=2)
                nc.sync.dma_start(out=dg, in_=conv_diag[ddt])
                shard = NT_DS <= ddt
                if shard:
                    xs = xssmp.tile([128, L], MMDT, tag="xssm", bufs=NT_DS)
                else:
                    xs = xssmp.tile([128, L], MMDT, tag="xtmp", bufs=2)
                x_ssm[ddt] = xs
                ptc = ps.tile([128, L], F32, tag="conv", bufs=1)
                for lc in range(NLC):
                    for k in range(K_CONV):
                        nc.tensor.matmul(
                            out=ptc[:, lc * 512:(lc + 1) * 512],
                            lhsT=dg[:, k],
                            rhs=cin[:, lc * 512 + k: lc * 512 + k + 512],
                            start=(k == 0), stop=(k == K_CONV - 1),
                        )
                nc.scalar.activation(out=xs, in_=ptc, func=AF.Silu,
                                     bias=cb_sb[:, ddt:ddt + 1])
                if DBG and ddt == NT_DS:
                    nc.sync.dma_start(out=dbg_xs[:, :], in_=xs.bitcast(F32))
                # interleaved x_proj accumulation (open psum group on pt_xp)
                for lc in range(NLC):
                    nc.tensor.matmul(
                        out=pt_xp[:, lc * 512:(lc + 1) * 512],
                        lhsT=wxp[ddt],
                        rhs=xs[:, lc * 512:(lc + 1) * 512],
                        start=(ddt == 0), stop=(ddt == NT_DF - 1),
                        skip_group_check=True,
                    )

            x_dbl = xdblp.tile([128, L], MMDT, tag="xdbl")
            nc.scalar.activation(out=x_dbl, in_=pt_xp, func=AF.Identity)
            if DBG:
                nc.sync.dma_start(out=dbg_xdbl[:, :], in_=x_dbl.bitcast(F32))
            # stash B/C rows to DRAM for partition-broadcast reload
            nc.sync.dma_start(out=bc_rows[:, :],
                              in_=x_dbl[DT_RANK:128, :].bitcast(F32))

            # ---------- Phase 2: in_proj z -> silu -> park in DRAM ----------
            wz = []
            for kt in range(NT_M):
                w = bigw.tile([128, DSH], MMDT, tag="big", bufs=NT_M)
                nc.sync.dma_start(out=w, in_=w_inz[kt])
                wz.append(w)
            zb_sb = wsmall.tile([128, NT_DS], F32, tag="zb")
            nc.sync.dma_start(out=zb_sb, in_=z_bias[:, :])
            for zt in range(NT_DS):
                pt = ps.tile([128, L], F32, tag="big", bufs=2)
                for lc in range(NLC):
                    for kt in range(NT_M):
                        nc.tensor.matmul(
                            out=pt[:, lc * 512:(lc + 1) * 512],
                            lhsT=wz[kt][:, zt * 128:(zt + 1) * 128],
                            rhs=unT[kt][:, lc * 512:(lc + 1) * 512],
                            start=(kt == 0), stop=(kt == NT_M - 1),
                        )
                zs = zstr.tile([128, L], F32, tag="z", bufs=2)
                nc.scalar.activation(out=zs, in_=pt, func=AF.Silu,
                                     bias=zb_sb[:, zt:zt + 1])
                nc.sync.dma_start(out=z_dram[zt], in_=zs)





            # ---------- Phase 3: dt = softplus(dt_raw @ dt_w.T + b) ----------
            wdt = wsmall.tile([DT_RANK, DSH], MMDT, tag="wdt")
            nc.sync.dma_start(out=wdt, in_=w_dt[:, :])
            dtb_sb = wsmall.tile([128, NT_DS], F32, tag="dtb")
            nc.sync.dma_start(out=dtb_sb, in_=dt_bias[:, :])
            a_sb = wsmall.tile([128, NT_DS, N_STATE], F32, tag="asc")
            nc.sync.dma_start(out=a_sb, in_=a_sc[:, :, :])
            d_sb = wsmall.tile([128, NT_DS], F32, tag="dvec")
            nc.sync.dma_start(out=d_sb, in_=d_vec[:, :])

            dts = []
            y0 = []
            for dtile in range(NT_DS):
                # fp32 softplus in a scratch tile, downcast to a persistent
                # bf16 dt (read only by the ACT abar exp), dtx in fp32 first
                dt32 = scratch.tile([128, L + K_CONV - 1], F32, tag="scr", bufs=4)
                dt32 = dt32[:, 0:L]
                pt = ps.tile([128, L], F32, tag="big", bufs=2)
                for lc in range(NLC):
                    nc.tensor.matmul(
                        out=pt[:, lc * 512:(lc + 1) * 512],
                        lhsT=wdt[:, dtile * 128:(dtile + 1) * 128],
                        rhs=x_dbl[0:DT_RANK, lc * 512:(lc + 1) * 512],
                        start=True, stop=True,
                    )
                # softplus(v + b) = ln(1 + exp(v + b))
                nc.scalar.activation(out=dt32, in_=pt, func=AF.Exp,
                                     bias=dtb_sb[:, dtile:dtile + 1])
                nc.scalar.activation(out=dt32, in_=dt32, func=AF.Ln, bias=1.0)
                if DBG and dtile == 0:
                    nc.sync.dma_start(out=dbg_dt[:, :], in_=dt32)

                xsf = x_ssm[NT_DS + dtile].bitcast(F32)
                t0 = ysp.tile([128, L], F32, tag="y", bufs=NT_DS + 1)
                nc.vector.tensor_scalar_mul(out=t0, in0=xsf,
                                            scalar1=d_sb[:, dtile:dtile + 1])
                y0.append(t0)
                # dtx overwrites the shard x tile (f32r-rounded output keeps
                # the BIR verifier happy; the extra ~1e-4 rounding is benign)
                nc.vector.tensor_tensor(out=x_ssm[NT_DS + dtile], in0=dt32,
                                        in1=xsf, op=OP.mult)

                dt_t = dtp.tile([128, L], mybir.dt.bfloat16, tag="dt", bufs=NT_DS)
                dts.append(dt_t)
                nc.vector.tensor_copy(out=dt_t, in_=dt32)

            # ---------- Phase 5: SSM scans, y accumulate ----------
            for n in range(N_STATE):
                bt = bcp.tile([128, L], F32, tag="bc", bufs=BC_BUFS)
                src = bc_rows[n:n + 1, :]
                src = bass.AP(tensor=src.tensor, offset=src.offset,
                              ap=[[0, 128]] + [list(d) for d in src.ap[1:]])
                nc.sync.dma_start(out=bt, in_=src)
                cdt = mybir.dt.bfloat16 if BF16_HC else F32
                ct = bcp.tile([128, L], cdt, tag="bc", bufs=BC_BUFS)
                src = bc_rows[N_STATE + n:N_STATE + n + 1, :]
                src = bass.AP(tensor=src.tensor, offset=src.offset,
                              ap=[[0, 128]] + [list(d) for d in src.ap[1:]])
                (nc.gpsimd if BF16_HC else nc.sync).dma_start(out=ct, in_=src)
                for dtile in range(NT_DS):
                    dtx = x_ssm[NT_DS + dtile].bitcast(F32)
                    abar = ps.tile([128, L], F32, tag="big", bufs=2)
                    nc.scalar.activation(out=abar, in_=dts[dtile], func=AF.Exp,
                                         scale=a_sb[:, dtile, n:n + 1])
                    bu = untp.tile([128, L], F32, tag="unT", bufs=NT_M)
                    eng = nc.vector if dtile < BU_DVE_TILES else nc.gpsimd
                    eng.tensor_tensor(out=bu, in0=dtx, in1=bt, op=OP.mult)
                    h = untp.tile([128, L], cdt, tag="unT", bufs=NT_M)
                    nc.vector.tensor_tensor_scan(out=h, data0=abar, data1=bu,
                                                 initial=0.0, op0=OP.mult, op1=OP.add)
                    tmp = bcp.tile([128, L], cdt, tag="tmp", bufs=TMP_BUFS)
                    nc.vector.tensor_tensor(out=tmp, in0=h, in1=ct, op=OP.mult)
                    if dtile < ACC_DVE_TILES:
                        nc.vector.tensor_tensor(out=y0[dtile], in0=y0[dtile],
                                                in1=tmp, op=OP.add)
                    else:
                        nc.gpsimd.dma_start(out=y0[dtile], in_=tmp, accum_op=OP.add)

            if DBG:
                nc.sync.dma_start(out=dbg_y[:, :], in_=y0[0])

            # ---------- Phase 6: gate in place + out_proj ----------
            y_g = []
            for dtile in range(NT_DS):
                zs = zstr.tile([128, L], F32, tag="z", bufs=2)
                nc.sync.dma_start(out=zs, in_=z_dram[dtile])
                g = ysp.tile([128, L], MMDT, tag="y", bufs=NT_DS + 1)
                nc.vector.tensor_tensor(out=g, in0=y0[dtile], in1=zs, op=OP.mult)
                y_g.append(g)

            wo = []
            for kt in range(NT_DS):
                w = bigw.tile([128, D_MODEL], MMDT, tag="big", bufs=NT_M)
                nc.sync.dma_start(out=w, in_=w_out[kt])
                wo.append(w)
            for mt in range(NT_M):
                otag, obufs = [("big", 2), ("conv", 1), ("xp", 1)][mt % 3]
                pt = ps.tile([128, L], F32, tag=otag, bufs=obufs)
                for lc in range(NLC):
                    for kt in range(NT_DS):
                        nc.tensor.matmul(
                            out=pt[:, lc * 512:(lc + 1) * 512],
                            lhsT=wo[kt][:, mt * 128:(mt + 1) * 128],
                            rhs=y_g[kt][:, lc * 512:(lc + 1) * 512],
                            start=(kt == 0), stop=(kt == NT_DS - 1),
                        )
                for lc in range(NLC):
                    ot = outsb.tile([128, 512], F32, tag="o", bufs=2)
                    nc.scalar.activation(out=ot, in_=pt[:, lc * 512:(lc + 1) * 512],
                                         func=AF.Identity)
                    nc.sync.dma_start(out=outp[mt][:, lc * 512:(lc + 1) * 512],
                                      in_=ot)

    nc.finalize()
    return nc



def _round_f32r(a):
    """Round fp32 array to float32r (RNE at 11 mantissa bits)."""
    if not USE_F32R:
        return np.ascontiguousarray(a, np.float32)
    a = np.ascontiguousarray(a, np.float32)
    bits = a.view(np.uint32).astype(np.uint64)
    drop = 12
    half = np.uint64(1 << (drop - 1))
    low = bits & np.uint64((1 << drop) - 1)
    out = bits + half
    tie = low == half
    out &= np.uint64(~((1 << drop) - 1) & 0xFFFFFFFF)
    lsb = np.uint64(1 << drop)
    out -= np.where(tie & ((out & lsb) != 0) & ((bits & lsb) == 0), lsb,
                    np.uint64(0))
    return out.astype(np.uint32).view(np.float32).reshape(a.shape)

def _shard_inputs(inputs):
    """Build the 8 per-core input maps. Core c: batch c>>2, branch (c>>1)&1,
    half c&1."""
    u = np.ascontiguousarray(np.asarray(inputs["u"], np.float32))
    norm_w = np.asarray(inputs["norm_w"], np.float32)
    norm_b = np.asarray(inputs["norm_b"], np.float32)

    in_maps = []
    meta = []
    for c in range(8):
        b, r, h = c >> 2, (c >> 1) & 1, c & 1
        pre = "fwd_" if r == 0 else "bwd_"
        in_w = np.asarray(inputs[pre + "in_w"], np.float32)
        conv_w = np.asarray(inputs[pre + "conv_w"], np.float32).reshape(D_INNER, K_CONV)
        conv_b = np.asarray(inputs[pre + "conv_b"], np.float32)
        A_log = np.asarray(inputs[pre + "A_log"], np.float32)
        xproj_w = np.asarray(inputs[pre + "xproj_w"], np.float32)
        dt_w = np.asarray(inputs[pre + "dt_w"], np.float32)
        dt_b = np.asarray(inputs[pre + "dt_b"], np.float32)
        D_p = np.asarray(inputs[pre + "D"], np.float32)
        out_w = np.asarray(inputs["out_w"], np.float32)

        sh = slice(h * DSH, (h + 1) * DSH)
        # channel order inside this core's program: non-shard half first,
        # the scanned shard last (the program scans x tiles 6..11)
        perm = np.r_[np.arange((1 - h) * DSH, (2 - h) * DSH),
                     np.arange(h * DSH, (h + 1) * DSH)]

        ub = u[b] if r == 0 else u[b, ::-1]
        u_t = np.ascontiguousarray(ub.reshape(NT_L, 128, D_MODEL))

        # fold norm affine into in_proj
        in_w_eff = in_w * norm_w[None, :]
        bias_full = in_w @ norm_b            # (2*D_INNER,)

        w_x = in_w_eff[:D_INNER][perm]       # (1536, 768), permuted
        w_z = in_w_eff[D_INNER:][sh]         # (768, 768)
        w_inx = _round_f32r(w_x.T.reshape(NT_M, 128, D_INNER))
        w_inz = _round_f32r(w_z.T.reshape(NT_M, 128, DSH))
        x_bias = np.ascontiguousarray(
            bias_full[:D_INNER][perm].reshape(NT_DF, 128).T)
        z_bias = np.ascontiguousarray(bias_full[D_INNER:][sh].reshape(NT_DS, 128).T)

        conv_w_p = conv_w[perm]
        cd = np.zeros((NT_DF, 128, K_CONV, 128), np.float32)
        idx = np.arange(128)
        for g in range(NT_DF):
            for k in range(K_CONV):
                cd[g, idx, k, idx] = conv_w_p[g * 128:(g + 1) * 128, k]
        cd = _round_f32r(cd)
        conv_bias = np.ascontiguousarray(conv_b[perm].reshape(NT_DF, 128).T)

        w_xp = _round_f32r(xproj_w[:, perm].T.reshape(NT_DF, 128, 128))
        w_dt_t = _round_f32r(dt_w[sh].T)              # (96, 768)
        dtb = np.ascontiguousarray(dt_b[sh].reshape(NT_DS, 128).T)
        A = -np.exp(A_log[sh].astype(np.float64)).astype(np.float32)   # (768, 16)
        a_sc = np.ascontiguousarray(A.reshape(NT_DS, 128, N_STATE).transpose(1, 0, 2))
        d_vec = np.ascontiguousarray(D_p[sh].reshape(NT_DS, 128).T)

        col = slice(r * D_INNER + h * DSH, r * D_INNER + (h + 1) * DSH)
        w_o = _round_f32r(out_w[:, col].T.reshape(NT_DS, 128, D_MODEL))

        in_maps.append({
            "u_in": u_t, "w_inx": w_inx, "w_inz": w_inz,
            "x_bias": x_bias, "z_bias": z_bias,
            "conv_diag": cd, "conv_b": conv_bias,
            "w_xproj": w_xp, "w_dt": w_dt_t, "dt_bias": dtb,
            "a_sc": a_sc, "d_vec": d_vec, "w_out": w_o,
        })
        meta.append((b, r, h))
    return in_maps, meta


def kernel(**inputs):
    global LAST_RESULTS
    nc = _build_program()
    in_maps, meta = _shard_inputs(inputs)
    trace = os.environ.get("KBENCH_TRACE", "0") == "1"
    res = run_bass_kernel_spmd(nc, in_maps, core_ids=list(range(8)), trace=trace)
    LAST_RESULTS = res

    u = np.asarray(inputs["u"], np.float32)
    out = np.array(u, np.float32, copy=True)
    for c, (b, r, h) in enumerate(meta):
        p = res.results[c]["outp"].reshape(D_MODEL, L).T   # (L, 768)
        if r == 1:
            p = p[::-1]
        out[b] += p
    return out



# revision 16
# speedup vs baseline: 1.3035x; 1.3035x over previous
"""BiMambaBlock Trainium2 kernel (v2).

Full inputs in, full output out. 8-way SPMD shard over (batch=2) x
(direction fwd/bwd) x (d_inner half). Each core runs the same Bass/Tile
program on its own data:

  LN (stats in time-major) -> PE transpose -> unT (bf16) -> in_proj x
  (full 1536) -> depthwise causal conv as 4 accumulating diagonal
  matmuls -> SiLU -> x_proj (interleaved psum accumulation) -> B/C rows
  parked in DRAM (bf16) and broadcast-loaded -> dt_proj + softplus ->
  scan phase with dtile OUTER / n INNER:
      abar = ACT exp(bf16), bu = DVE bf16 mult, h = scan (Pool/DVE),
      tmp = DVE bf16 mult, y accumulated on the PE into PSUM via
      identity matmuls (seeded with diag(D) @ x), gate vs silu(z)
      in-place, out_proj partials accumulated at the tail.

The bwd direction is the same program on a time-reversed copy of u; its
partial output is reversed back on the host. The 4 partials per batch
are summed on the host (row-parallel out_proj) and the residual added.
"""

import os
import numpy as np
import ml_dtypes

import concourse.bass as bass
import concourse.bacc as bacc
import concourse.hw_specs as hw_specs

_orig_get_tables = hw_specs.get_activation_tables


def _tables_nlx_first(arch):
    """Keep canonical table order/indices (walrus interprets act_func_set_id
    positionally) but blank the exp-only / ln-only tables so the load
    chooser resolves Exp and Ln to the combined natural_log_exp table."""
    tabs = _orig_get_tables(arch)
    out = {}
    for k, v in tabs.items():
        out[k] = set() if k in ("exp_and_others", "natural_log",
                                "exp_and_friends") else v
    return out


hw_specs.get_activation_tables = _tables_nlx_first
bacc.get_activation_tables = _tables_nlx_first
import concourse.mybir as mybir
import concourse.tile as tile
from concourse.bass_utils import run_bass_kernel_spmd

F32 = mybir.dt.float32
BF16 = mybir.dt.bfloat16
AF = mybir.ActivationFunctionType
OP = mybir.AluOpType

D_MODEL = 768
D_INNER = 1536
N_STATE = 16
DT_RANK = 96
K_CONV = 4
B, L = 2, 1024
DSH = D_INNER // 2          # 768 channels scanned per core
NT_M = D_MODEL // 128       # 6 tiles of model dim
NT_DF = D_INNER // 128      # 12 tiles of full d_inner
NT_DS = DSH // 128          # 6 tiles of the scan shard
NT_L = L // 128             # 8 time tiles
NLC = L // 512              # 2 psum column chunks

# scans are DVE-only (walrus rejects Pool TensorScalarPtr); Pool absorbs
# a slice of the bu/tmp multiplies to balance the two engines.
ACC_PE = os.environ.get("KERNEL_ACC", "pe") == "pe"     # y acc via PE matmul
GATE_POOL = os.environ.get("KERNEL_GATE", "dve") == "pool"
POOL_BU_N = frozenset(
    int(v) for v in os.environ.get("KERNEL_POOL_BU_N", "1,3,5,7,9,11,13,15").split(",") if v)
POOL_TMP_N = frozenset(
    int(v) for v in os.environ.get("KERNEL_POOL_TMP_N", "0,4,8,12,2,6").split(",") if v)
BC_BATCH = os.environ.get("KERNEL_BC_BATCH", "1") == "1"
UNT3D = os.environ.get("KERNEL_UNT3D", "1") == "1"
STOP_AFTER = int(os.environ.get("KERNEL_STOP_AFTER", "9"))  # bisect knob

LAST_RESULTS = None  # BassKernelResults stash for test.py


def _build_program():
    nc = bacc.Bacc("TRN2", target_bir_lowering=False)

    # ---- DRAM I/O (per-core shapes) ----
    u_in = nc.dram_tensor("u_in", [NT_L, 128, D_MODEL], F32, kind="ExternalInput")
    w_inx = nc.dram_tensor("w_inx", [NT_M, 128, D_INNER], BF16, kind="ExternalInput")
    w_inz = nc.dram_tensor("w_inz", [NT_M, 128, DSH], BF16, kind="ExternalInput")
    x_bias = nc.dram_tensor("x_bias", [128, NT_DF], F32, kind="ExternalInput")
    z_bias = nc.dram_tensor("z_bias", [128, NT_DS], F32, kind="ExternalInput")
    conv_diag = nc.dram_tensor("conv_diag", [NT_DF, 128, K_CONV, 128], BF16,
                               kind="ExternalInput")
    conv_b = nc.dram_tensor("conv_b", [128, NT_DF], F32, kind="ExternalInput")
    w_xproj = nc.dram_tensor("w_xproj", [NT_DF, 128, 128], BF16, kind="ExternalInput")
    w_dt = nc.dram_tensor("w_dt", [DT_RANK, DSH], BF16, kind="ExternalInput")
    dt_bias = nc.dram_tensor("dt_bias", [128, NT_DS], F32, kind="ExternalInput")
    a_sc = nc.dram_tensor("a_sc", [128, NT_DS, N_STATE], F32, kind="ExternalInput")
    d_diag = nc.dram_tensor("d_diag", [NT_DS, 128, 128], BF16, kind="ExternalInput")
    w_out = nc.dram_tensor("w_out", [NT_DS, 128, D_MODEL], BF16, kind="ExternalInput")
    outp = nc.dram_tensor("outp", [NT_M, 128, L], F32, kind="ExternalOutput")

    with tile.TileContext(nc) as tc:
        with (
            tc.tile_pool(name="bigw", bufs=1) as bigw,
            tc.tile_pool(name="wsmall", bufs=1) as wsmall,
            tc.tile_pool(name="wstream", bufs=1) as wstream,
            tc.tile_pool(name="scratch", bufs=1) as scratch,
            tc.tile_pool(name="stat", bufs=4) as stat,
            tc.tile_pool(name="unt", bufs=1) as untp,
            tc.tile_pool(name="xssm", bufs=1) as xssmp,
            tc.tile_pool(name="dts", bufs=1) as dtp,
            tc.tile_pool(name="zg", bufs=1) as zgp,
            tc.tile_pool(name="xdbl", bufs=1) as xdblp,
            tc.tile_pool(name="bcp", bufs=1) as bcp,
            tc.tile_pool(name="rot", bufs=1) as rotp,
            tc.tile_pool(name="ps", bufs=1, space="PSUM") as ps,
            tc.tile_pool(name="drp", bufs=1, space="DRAM") as drp,
        ):
            bc_dram = drp.tile([2 * N_STATE, L], BF16, tag="bcr")
            # identities for PE transposes / accumulation (fill where compare FAILS)
            ident = wsmall.tile([128, 128], F32, tag="ident")
            nc.vector.memset(ident, 0.0)
            nc.gpsimd.affine_select(
                out=ident, in_=ident, compare_op=OP.not_equal, fill=1.0,
                base=0, pattern=[[-1, 128]], channel_multiplier=1,
            )
            ident_bf = wsmall.tile([128, 128], BF16, tag="identbf")
            nc.vector.tensor_copy(out=ident_bf, in_=ident)

            # ---------- Phase 0: LayerNorm (time-major) -> unT (bf16) -----
            eps = stat.tile([128, 1], F32, tag="eps", bufs=1)
            nc.vector.memset(eps, 1e-5)
            unT = untp.tile([128, NT_M * L], BF16, tag="unT")
            unTv = unT.rearrange("p (m t) -> p m t", m=NT_M)
            for lt in range(NT_L):
                ut = scratch.tile([128, L + K_CONV - 1], F32, tag="scr", bufs=2)
                nc.sync.dma_start(out=ut[:, 0:D_MODEL], in_=u_in[lt])
                sub = ut[:, 0:D_MODEL].rearrange("p (s f) -> p s f", f=256)
                st = stat.tile([128, 3, nc.vector.BN_STATS_DIM], F32, tag="bst")
                for sg in range(3):
                    nc.vector.bn_stats(out=st[:, sg], in_=sub[:, sg])
                mv = stat.tile([128, nc.vector.BN_AGGR_DIM], F32, tag="mv")
                nc.vector.bn_aggr(out=mv, in_=st)
                mean = mv[:, 0:1]
                rstd = stat.tile([128, 1], F32, tag="rstd")
                nc.scalar.activation(out=rstd, in_=mv[:, 1:2], func=AF.Sqrt,
                                     bias=eps[:, 0:1])
                nc.vector.reciprocal(out=rstd, in_=rstd)
                nc.vector.tensor_scalar(out=ut[:, 0:D_MODEL], in0=ut[:, 0:D_MODEL],
                                        scalar1=mean, scalar2=rstd,
                                        op0=OP.subtract, op1=OP.mult)
                pt = ps.tile([128, D_MODEL], F32, tag="big", bufs=2)
                for mt in range(NT_M):
                    nc.tensor.transpose(
                        out=pt[:, mt * 128:(mt + 1) * 128],
                        in_=ut[:, mt * 128:(mt + 1) * 128],
                        identity=ident,
                    )
                # one batched copy into all 6 unT column blocks
                if UNT3D:
                    ptv = pt.rearrange("p (m f) -> p m f", m=NT_M)
                    nc.scalar.activation(
                        out=unTv[:, :, lt * 128:(lt + 1) * 128],
                        in_=ptv, func=AF.Identity)
                else:
                    for mt in range(NT_M):
                        nc.scalar.activation(
                            out=unTv[:, mt, lt * 128:(lt + 1) * 128],
                            in_=pt[:, mt * 128:(mt + 1) * 128],
                            func=AF.Identity)

            # ---------- Phase 1: in_proj x -> conv -> silu, x_proj ---------
            wx = []
            for kt in range(NT_M):
                w = bigw.tile([128, D_INNER], BF16, tag="big", bufs=NT_M)
                nc.sync.dma_start(out=w, in_=w_inx[kt])
                wx.append(w)
            wxp = []
            for kt in range(NT_DF):
                w = wsmall.tile([128, 128], BF16, tag=f"wxp{kt}")
                nc.sync.dma_start(out=w, in_=w_xproj[kt])
                wxp.append(w)

            xb_sb = wsmall.tile([128, NT_DF], F32, tag="xb")
            nc.sync.dma_start(out=xb_sb, in_=x_bias[:, :])
            cb_sb = wsmall.tile([128, NT_DF], F32, tag="cb")
            nc.sync.dma_start(out=cb_sb, in_=conv_b[:, :])

            zpad32 = wsmall.tile([128, K_CONV - 1], F32, tag="zpad32")
            nc.vector.memset(zpad32, 0.0)
            zpad = wsmall.tile([128, K_CONV - 1], BF16, tag="zpad")
            nc.vector.tensor_copy(out=zpad, in_=zpad32)

            pt_xp = ps.tile([128, L], F32, tag="xp", bufs=1)
            x_ssm = [None] * NT_DF
            for ddt in range(NT_DF):
                cin = scratch.tile([128, L + K_CONV - 1], BF16, tag="scrb",
                                   bufs=3)
                nc.vector.tensor_copy(out=cin[:, 0:K_CONV - 1], in_=zpad)
                pt = ps.tile([128, L], F32, tag="big", bufs=2)
                for lc in range(NLC):
                    for kt in range(NT_M):
                        nc.tensor.matmul(
                            out=pt[:, lc * 512:(lc + 1) * 512],
                            lhsT=wx[kt][:, ddt * 128:(ddt + 1) * 128],
                            rhs=unTv[:, kt, lc * 512:(lc + 1) * 512],
                            start=(kt == 0), stop=(kt == NT_M - 1),
                        )
                nc.scalar.activation(out=cin[:, K_CONV - 1:], in_=pt,
                                     func=AF.Identity, bias=xb_sb[:, ddt:ddt + 1])
                dg = wstream.tile([128, K_CONV, 128], BF16, tag="diag", bufs=2)
                nc.sync.dma_start(out=dg, in_=conv_diag[ddt])
                shard = NT_DS <= ddt
                if shard:
                    xs = xssmp.tile([128, L], BF16, tag="xssm", bufs=NT_DS)
                else:
                    xs = xssmp.tile([128, L], BF16, tag="xtmp", bufs=2)
                x_ssm[ddt] = xs
                ptc = ps.tile([128, L], F32, tag="conv", bufs=1)
                for lc in range(NLC):
                    for k in range(K_CONV):
                        nc.tensor.matmul(
                            out=ptc[:, lc * 512:(lc + 1) * 512],
                            lhsT=dg[:, k],
                            rhs=cin[:, lc * 512 + k: lc * 512 + k + 512],
                            start=(k == 0), stop=(k == K_CONV - 1),
                        )
                nc.scalar.activation(out=xs, in_=ptc, func=AF.Silu,
                                     bias=cb_sb[:, ddt:ddt + 1])
                # interleaved x_proj accumulation (open psum group on pt_xp)
                for lc in range(NLC):
                    nc.tensor.matmul(
                        out=pt_xp[:, lc * 512:(lc + 1) * 512],
                        lhsT=wxp[ddt],
                        rhs=xs[:, lc * 512:(lc + 1) * 512],
                        start=(ddt == 0), stop=(ddt == NT_DF - 1),
                        skip_group_check=True,
                    )

            x_dbl = xdblp.tile([128, L], BF16, tag="xdbl")
            nc.scalar.activation(out=x_dbl, in_=pt_xp, func=AF.Identity)
            # park B/C rows (bf16) for partition-broadcast reload
            if STOP_AFTER >= 2:
                nc.sync.dma_start(out=bc_dram[:, :], in_=x_dbl[DT_RANK:128, :])

            if STOP_AFTER <= 1:
                xout = scratch.tile([128, L], F32, tag="scr", bufs=2, name="xo")
                nc.scalar.activation(out=xout, in_=x_dbl, func=AF.Identity)
                nc.sync.dma_start(out=outp[0][:, :], in_=xout)

            # resident broadcast B/C tiles
            bt_all = bcp.tile([128, N_STATE, L], BF16, tag="bta")
            ct_all = bcp.tile([128, N_STATE, L], BF16, tag="cta")
            if STOP_AFTER < 2:
                pass
            elif BC_BATCH:
                for half, dst in ((0, bt_all), (1, ct_all)):
                    for ch in range(0, N_STATE, 4):
                        src = bc_dram[half * N_STATE + ch: half * N_STATE + ch + 4, :]
                        src = bass.AP(tensor=src.tensor, offset=src.offset,
                                      ap=[[0, 128]] + [list(d) for d in src.ap],
                                      )
                        nc.sync.dma_start(out=dst[:, ch:ch + 4], in_=src)
            else:
                for half, dst in ((0, bt_all), (1, ct_all)):
                    for n in range(N_STATE):
                        src = bc_dram[half * N_STATE + n: half * N_STATE + n + 1, :]
                        src = bass.AP(tensor=src.tensor, offset=src.offset,
                                      ap=[[0, 128]] + [list(d) for d in src.ap[1:]])
                        nc.sync.dma_start(out=dst[:, n], in_=src)
            bts = [bt_all[:, n] for n in range(N_STATE)]
            cts = [ct_all[:, n] for n in range(N_STATE)]

            # ---------- Phase 3: dt = softplus(dt_raw @ dt_w.T + b) -------
            wdt = wsmall.tile([DT_RANK, DSH], BF16, tag="wdt")
            nc.sync.dma_start(out=wdt, in_=w_dt[:, :])
            dtb_sb = wsmall.tile([128, NT_DS], F32, tag="dtb")
            nc.sync.dma_start(out=dtb_sb, in_=dt_bias[:, :])
            a_sb = wsmall.tile([128, NT_DS, N_STATE], F32, tag="asc")
            nc.sync.dma_start(out=a_sb, in_=a_sc[:, :, :])
            zb_sb = wsmall.tile([128, NT_DS], F32, tag="zb")
            nc.sync.dma_start(out=zb_sb, in_=z_bias[:, :])
            dd_sb = []
            for dtile in range(NT_DS):
                dd = wstream.tile([128, 128], BF16, tag="ddiag", bufs=NT_DS)
                nc.sync.dma_start(out=dd, in_=d_diag[dtile])
                dd_sb.append(dd)
            wz = []
            for kt in range(NT_M):
                w = wstream.tile([128, DSH], BF16, tag="wz", bufs=NT_M)
                nc.sync.dma_start(out=w, in_=w_inz[kt])
                wz.append(w)

            dts = []
            dtxs = []
            e16s = []
            for dtile in range(NT_DS if STOP_AFTER >= 2 else 0):
                pt = ps.tile([128, L], F32, tag="conv", bufs=1)
                for lc in range(NLC):
                    nc.tensor.matmul(
                        out=pt[:, lc * 512:(lc + 1) * 512],
                        lhsT=wdt[:, dtile * 128:(dtile + 1) * 128],
                        rhs=x_dbl[0:DT_RANK, lc * 512:(lc + 1) * 512],
                        start=True, stop=True,
                    )
                # softplus = ln(1 + exp(v + b)); exp parked bf16 in rot rings
                etag = ["abar", "bu"][dtile % 2]
                e16 = rotp.tile([128, L], BF16, tag=etag, bufs=3, name=f"e16_{dtile}")
                nc.scalar.activation(out=e16, in_=pt, func=AF.Exp,
                                     bias=dtb_sb[:, dtile:dtile + 1])
                e16s.append(e16)
            for dtile in range(len(e16s)):
                dt_t = dtp.tile([128, L], BF16, tag="dt", bufs=NT_DS)
                nc.scalar.activation(out=dt_t, in_=e16s[dtile], func=AF.Ln,
                                     bias=1.0)
                dts.append(dt_t)
                dtx = dtp.tile([128, L], BF16, tag="dtx", bufs=NT_DS)
                nc.vector.tensor_tensor(out=dtx, in0=dt_t,
                                        in1=x_ssm[NT_DS + dtile], op=OP.mult)
                dtxs.append(dtx)

            if STOP_AFTER == 2:
                xout = scratch.tile([128, L], F32, tag="scr", bufs=2, name="xo")
                nc.scalar.activation(out=xout, in_=dtxs[0], func=AF.Identity)
                nc.sync.dma_start(out=outp[0][:, :], in_=xout)

            # ---------- Phase 5: z matmuls (silu deferred), scan loop ------
            z_psum = []
            run5 = STOP_AFTER >= 3
            for zt in range(NT_DS if run5 else 0):
                ptz = ps.tile([128, L], F32, tag=["xp", "conv"][zt % 2], bufs=1,
                              name=f"ptz{zt}")
                for lc in range(NLC):
                    for kt in range(NT_M):
                        nc.tensor.matmul(
                            out=ptz[:, lc * 512:(lc + 1) * 512],
                            lhsT=wz[kt][:, zt * 128:(zt + 1) * 128],
                            rhs=unTv[:, kt, lc * 512:(lc + 1) * 512],
                            start=(kt == 0), stop=(kt == NT_M - 1),
                            skip_group_check=True,
                        )
                z_psum.append(ptz)

            z_tiles = [None] * NT_DS
            g_tiles = []
            for dtile in range(NT_DS if run5 else 0):
                y0ps = ps.tile([128, L], F32, tag="big", bufs=2)
                xsf = x_ssm[NT_DS + dtile]
                # seed y = D * x via diagonal matmul
                for lc in range(NLC):
                    nc.tensor.matmul(
                        out=y0ps[:, lc * 512:(lc + 1) * 512],
                        lhsT=dd_sb[dtile],
                        rhs=xsf[:, lc * 512:(lc + 1) * 512],
                        start=True, stop=False,
                        skip_group_check=True,
                    )
                for n in range(N_STATE):
                    abar = rotp.tile([128, L], BF16, tag="abar", bufs=3)
                    nc.scalar.activation(out=abar, in_=dts[dtile], func=AF.Exp,
                                         scale=a_sb[:, dtile, n:n + 1])
                    bu = rotp.tile([128, L], BF16, tag="bu", bufs=3)
                    bu_eng = nc.gpsimd if n in POOL_BU_N else nc.vector
                    bu_eng.tensor_tensor(out=bu, in0=dtxs[dtile], in1=bts[n],
                                         op=OP.mult)
                    h = rotp.tile([128, L], BF16, tag="h", bufs=3)
                    nc.vector.tensor_tensor_scan(out=h, data0=abar, data1=bu,
                                                 initial=0.0, op0=OP.mult,
                                                 op1=OP.add)
                    tmp = rotp.tile([128, L], BF16, tag="tmp", bufs=3)
                    tmp_eng = nc.gpsimd if n in POOL_TMP_N else nc.vector
                    tmp_eng.tensor_tensor(out=tmp, in0=h, in1=cts[n],
                                          op=OP.mult)
                    if ACC_PE:
                        last = n == N_STATE - 1
                        for lc in range(NLC):
                            nc.tensor.matmul(
                                out=y0ps[:, lc * 512:(lc + 1) * 512],
                                lhsT=ident_bf,
                                rhs=tmp[:, lc * 512:(lc + 1) * 512],
                                start=False, stop=last,
                                skip_group_check=True,
                            )
                    else:
                        nc.vector.tensor_tensor(out=y0ps, in0=y0ps, in1=tmp,
                                                op=OP.add)
                if dtile == 0:
                    for zt in range(NT_DS):
                        zt_t = zgp.tile([128, L], BF16, tag="z", bufs=NT_DS,
                                        name=f"zs{zt}")
                        nc.scalar.activation(out=zt_t, in_=z_psum[zt],
                                             func=AF.Silu,
                                             bias=zb_sb[:, zt:zt + 1])
                        z_tiles[zt] = zt_t
                # gate in place: g = y * silu(z) (zs tile becomes g)
                zs = z_tiles[dtile]
                gate_eng = nc.gpsimd if GATE_POOL else nc.vector
                gate_eng.tensor_tensor(out=zs, in0=y0ps, in1=zs, op=OP.mult)
                g_tiles.append(zs)

            if STOP_AFTER == 3:
                xout = scratch.tile([128, L], F32, tag="scr", bufs=2, name="xo")
                nc.scalar.activation(out=xout, in_=g_tiles[0], func=AF.Identity)
                nc.sync.dma_start(out=outp[0][:, :], in_=xout)

            # ---------- Phase 6: out_proj ----------
            run6 = STOP_AFTER >= 4
            wo = []
            for kt in range(NT_DS if run6 else 0):
                w = bigw.tile([128, D_MODEL], BF16, tag="big", bufs=NT_M)
                nc.sync.dma_start(out=w, in_=w_out[kt])
                wo.append(w)
            for mt in range(NT_M if run6 else 0):
                otag, obufs = [("big", 2), ("conv", 1), ("xp", 1)][mt % 3]
                pt = ps.tile([128, L], F32, tag=otag, bufs=obufs)
                for lc in range(NLC):
                    for kt in range(NT_DS):
                        nc.tensor.matmul(
                            out=pt[:, lc * 512:(lc + 1) * 512],
                            lhsT=wo[kt][:, mt * 128:(mt + 1) * 128],
                            rhs=g_tiles[kt][:, lc * 512:(lc + 1) * 512],
                            start=(kt == 0), stop=(kt == NT_DS - 1),
                        )
                for lc in range(NLC):
                    ot = dtp.tile([128, 512], F32, tag="dtx", bufs=NT_DS,
                                  name="ot")
                    nc.scalar.activation(out=ot, in_=pt[:, lc * 512:(lc + 1) * 512],
                                         func=AF.Identity)
                    nc.sync.dma_start(out=outp[mt][:, lc * 512:(lc + 1) * 512],
                                      in_=ot)

    nc.finalize()
    return nc


def _bf16(a):
    return np.ascontiguousarray(np.asarray(a, np.float32)).astype(ml_dtypes.bfloat16)


def _shard_inputs(inputs):
    """Build the 8 per-core input maps. Core c: batch c>>2, branch (c>>1)&1,
    half c&1."""
    u = np.ascontiguousarray(np.asarray(inputs["u"], np.float32))
    norm_w = np.asarray(inputs["norm_w"], np.float32)
    norm_b = np.asarray(inputs["norm_b"], np.float32)

    in_maps = []
    meta = []
    for c in range(8):
        b, r, h = c >> 2, (c >> 1) & 1, c & 1
        pre = "fwd_" if r == 0 else "bwd_"
        in_w = np.asarray(inputs[pre + "in_w"], np.float32)
        conv_w = np.asarray(inputs[pre + "conv_w"], np.float32).reshape(D_INNER, K_CONV)
        conv_b = np.asarray(inputs[pre + "conv_b"], np.float32)
        A_log = np.asarray(inputs[pre + "A_log"], np.float32)
        xproj_w = np.asarray(inputs[pre + "xproj_w"], np.float32)
        dt_w = np.asarray(inputs[pre + "dt_w"], np.float32)
        dt_b = np.asarray(inputs[pre + "dt_b"], np.float32)
        D_p = np.asarray(inputs[pre + "D"], np.float32)
        out_w = np.asarray(inputs["out_w"], np.float32)

        sh = slice(h * DSH, (h + 1) * DSH)
        # channel order inside this core's program: non-shard half first,
        # the scanned shard last (the program scans x tiles 6..11)
        perm = np.r_[np.arange((1 - h) * DSH, (2 - h) * DSH),
                     np.arange(h * DSH, (h + 1) * DSH)]

        ub = u[b] if r == 0 else u[b, ::-1]
        u_t = np.ascontiguousarray(ub.reshape(NT_L, 128, D_MODEL))

        # fold norm affine into in_proj
        in_w_eff = in_w * norm_w[None, :]
        bias_full = in_w @ norm_b            # (2*D_INNER,)

        w_x = in_w_eff[:D_INNER][perm]       # (1536, 768), permuted
        w_z = in_w_eff[D_INNER:][sh]         # (768, 768)
        w_inx = _bf16(w_x.T.reshape(NT_M, 128, D_INNER))
        w_inz = _bf16(w_z.T.reshape(NT_M, 128, DSH))
        x_bias = np.ascontiguousarray(
            bias_full[:D_INNER][perm].reshape(NT_DF, 128).T)
        z_bias = np.ascontiguousarray(bias_full[D_INNER:][sh].reshape(NT_DS, 128).T)

        conv_w_p = conv_w[perm]
        cd = np.zeros((NT_DF, 128, K_CONV, 128), np.float32)
        idx = np.arange(128)
        for g in range(NT_DF):
            for k in range(K_CONV):
                cd[g, idx, k, idx] = conv_w_p[g * 128:(g + 1) * 128, k]
        cd = _bf16(cd)
        conv_bias = np.ascontiguousarray(conv_b[perm].reshape(NT_DF, 128).T)

        w_xp = _bf16(xproj_w[:, perm].T.reshape(NT_DF, 128, 128))
        w_dt_t = _bf16(dt_w[sh].T)              # (96, 768)
        dtb = np.ascontiguousarray(dt_b[sh].reshape(NT_DS, 128).T)
        A = -np.exp(A_log[sh].astype(np.float64)).astype(np.float32)   # (768, 16)
        a_sc = np.ascontiguousarray(A.reshape(NT_DS, 128, N_STATE).transpose(1, 0, 2))
        dd = np.zeros((NT_DS, 128, 128), np.float32)
        Dsh = D_p[sh]
        for g in range(NT_DS):
            dd[g, idx, idx] = Dsh[g * 128:(g + 1) * 128]
        dd = _bf16(dd)

        col = slice(r * D_INNER + h * DSH, r * D_INNER + (h + 1) * DSH)
        w_o = _bf16(out_w[:, col].T.reshape(NT_DS, 128, D_MODEL))

        in_maps.append({
            "u_in": u_t, "w_inx": w_inx, "w_inz": w_inz,
            "x_bias": x_bias, "z_bias": z_bias,
            "conv_diag": cd, "conv_b": conv_bias,
            "w_xproj": w_xp, "w_dt": w_dt_t, "dt_bias": dtb,
            "a_sc": a_sc, "d_diag": dd, "w_out": w_o,
        })
        meta.append((b, r, h))
    return in_maps, meta


def kernel(**inputs):
    global LAST_RESULTS
    nc = _build_program()
    in_maps, meta = _shard_inputs(inputs)
    trace = os.environ.get("KBENCH_TRACE", "0") == "1"
    res = run_bass_kernel_spmd(nc, in_maps, core_ids=list(range(8)), trace=trace)
    LAST_RESULTS = res

    u = np.asarray(inputs["u"], np.float32)
    out = np.array(u, np.float32, copy=True)
    for c, (b, r, h) in enumerate(meta):
        p = np.asarray(res.results[c]["outp"], np.float32).reshape(D_MODEL, L).T
        if r == 1:
            p = p[::-1]
        out[b] += p
    return out


# revision 20
# speedup vs baseline: 1.4990x; 1.1500x over previous
"""BiMambaBlock Trainium2 kernel (v3, fp8 front-end).

Full inputs in, full output out. 8-way SPMD shard over (batch=2) x
(direction fwd/bwd) x (d_inner half). Per core:

  LN (stats time-major) -> PE transpose -> unT (fp8e4) -> in_proj x as
  fp8 DoubleRow matmuls (weights prescaled x64, undone in the ACT copy)
  -> depthwise causal conv as fp8 DoubleRow diagonal matmuls over tap
  pairs (weights x16) -> SiLU -> x_proj (bf16, interleaved psum
  accumulation) -> B/C rows parked in DRAM bf16 + broadcast-loaded ->
  dt softplus (exp+ln, one act table) -> scan phase, dtile outer:
     abar = ACT exp bf16; bu = dtx*B (DVE/Pool bf16); h = DVE scan;
     tmp = h*C (DVE/Pool); y accumulated on PE into PSUM via identity
     matmuls, seeded with diag(D) @ x; gate g = (8*y)*silu(z) -> fp8
  -> out_proj as fp8 DoubleRow (weights x64; output scaled 1/512).

The bwd direction runs the same program on time-reversed u; partials
are reversed/summed on the host with the residual.
"""

import os
import numpy as np
import ml_dtypes

import concourse.bass as bass
import concourse.bacc as bacc
import concourse.hw_specs as hw_specs

_orig_get_tables = hw_specs.get_activation_tables


def _tables_nlx_first(arch):
    """Keep canonical table order/indices (walrus interprets
    act_func_set_id positionally) but blank the exp-only / ln-only tables
    so the chooser resolves Exp and Ln to the combined table."""
    tabs = _orig_get_tables(arch)
    return {k: (set() if k in ("exp_and_others", "natural_log",
                               "exp_and_friends") else v)
            for k, v in tabs.items()}


hw_specs.get_activation_tables = _tables_nlx_first
bacc.get_activation_tables = _tables_nlx_first
import concourse.mybir as mybir
import concourse.tile as tile
from concourse.bass_utils import run_bass_kernel_spmd

F32 = mybir.dt.float32
BF16 = mybir.dt.bfloat16
FP8 = mybir.dt.float8e4
AF = mybir.ActivationFunctionType
OP = mybir.AluOpType
DR = mybir.MatmulPerfMode.DoubleRow

D_MODEL = 768
D_INNER = 1536
N_STATE = 16
DT_RANK = 96
K_CONV = 4
B, L = 2, 1024
DSH = D_INNER // 2          # 768 channels scanned per core
NT_M = D_MODEL // 128       # 6 tiles of model dim
NT_DF = D_INNER // 128      # 12 tiles of full d_inner
NT_DS = DSH // 128          # 6 tiles of the scan shard
NT_L = L // 128             # 8 time tiles
NLC = L // 512              # 2 psum column chunks

WXS = 64.0                  # in_proj weight prescale (fp8 subnormal dodge)
WCS = 16.0                  # conv weight prescale
WOS = 64.0                  # out_proj weight prescale
GS = 2.0                    # gate output prescale

# scans are DVE-only (walrus rejects Pool TensorScalarPtr); Pool absorbs
# a slice of the bu/tmp multiplies to balance the two engines.
POOL_BU_N = frozenset(
    int(v) for v in os.environ.get(
        "KERNEL_POOL_BU_N", "1,3,5,7,9,11,13,15").split(",") if v)
POOL_TMP_N = frozenset(
    int(v) for v in os.environ.get(
        "KERNEL_POOL_TMP_N", "0,4,8,12,2,6").split(",") if v)

LAST_RESULTS = None  # BassKernelResults stash for test.py


def _build_program():
    nc = bacc.Bacc("TRN2", target_bir_lowering=False)

    # ---- DRAM I/O (per-core shapes) ----
    u_in = nc.dram_tensor("u_in", [NT_L, 128, D_MODEL], F32, kind="ExternalInput")
    w_inx = nc.dram_tensor("w_inx", [128, NT_M, D_INNER], FP8, kind="ExternalInput")
    w_inz = nc.dram_tensor("w_inz", [128, NT_M, DSH], FP8, kind="ExternalInput")
    x_bias = nc.dram_tensor("x_bias", [128, NT_DF], F32, kind="ExternalInput")
    z_bias = nc.dram_tensor("z_bias", [128, NT_DS], F32, kind="ExternalInput")
    conv_diag = nc.dram_tensor("conv_diag", [NT_DF, 128, K_CONV, 128], FP8,
                               kind="ExternalInput")
    conv_b = nc.dram_tensor("conv_b", [128, NT_DF], F32, kind="ExternalInput")
    w_xproj = nc.dram_tensor("w_xproj", [NT_DF, 128, 128], BF16, kind="ExternalInput")
    w_dt = nc.dram_tensor("w_dt", [DT_RANK, DSH], BF16, kind="ExternalInput")
    dt_bias = nc.dram_tensor("dt_bias", [128, NT_DS], F32, kind="ExternalInput")
    a_sc = nc.dram_tensor("a_sc", [128, NT_DS, N_STATE], F32, kind="ExternalInput")
    d_diag = nc.dram_tensor("d_diag", [NT_DS, 128, 128], BF16, kind="ExternalInput")
    w_out = nc.dram_tensor("w_out", [128, NT_DS, D_MODEL], FP8, kind="ExternalInput")
    outp = nc.dram_tensor("outp", [NT_M, 128, L], F32, kind="ExternalOutput")

    with tile.TileContext(nc) as tc:
        with (
            tc.tile_pool(name="bigw", bufs=1) as bigw,
            tc.tile_pool(name="wsmall", bufs=1) as wsmall,
            tc.tile_pool(name="wstream", bufs=1) as wstream,
            tc.tile_pool(name="scratch", bufs=1) as scratch,
            tc.tile_pool(name="stat", bufs=4) as stat,
            tc.tile_pool(name="unt", bufs=1) as untp,
            tc.tile_pool(name="xssm", bufs=1) as xssmp,
            tc.tile_pool(name="dts", bufs=1) as dtp,
            tc.tile_pool(name="zg", bufs=1) as zgp,
            tc.tile_pool(name="xdbl", bufs=1) as xdblp,
            tc.tile_pool(name="bcp", bufs=1) as bcp,
            tc.tile_pool(name="rot", bufs=1) as rotp,
            tc.tile_pool(name="ps", bufs=1, space="PSUM") as ps,
            tc.tile_pool(name="drp", bufs=1, space="DRAM") as drp,
        ):
            bc_dram = drp.tile([2 * N_STATE, L], BF16, tag="bcr")
            # identities for PE transposes / accumulation
            ident = wsmall.tile([128, 128], F32, tag="ident")
            nc.vector.memset(ident, 0.0)
            nc.gpsimd.affine_select(
                out=ident, in_=ident, compare_op=OP.not_equal, fill=1.0,
                base=0, pattern=[[-1, 128]], channel_multiplier=1,
            )
            ident_bf = wsmall.tile([128, 128], BF16, tag="identbf")
            nc.vector.tensor_copy(out=ident_bf, in_=ident)

            # ---------- Phase 0: LayerNorm (time-major) -> unT (fp8) ------
            eps = stat.tile([128, 1], F32, tag="eps", bufs=1)
            nc.vector.memset(eps, 1e-5)
            unT = untp.tile([128, NT_M * L], FP8, tag="unT")
            unTv = unT.rearrange("p (m t) -> p m t", m=NT_M)
            for lt in range(NT_L):
                ut = scratch.tile([128, D_MODEL], F32, tag="ut", bufs=3)
                nc.sync.dma_start(out=ut, in_=u_in[lt])
                sub = ut.rearrange("p (s f) -> p s f", f=256)
                st = stat.tile([128, 3, nc.vector.BN_STATS_DIM], F32, tag="bst")
                for sg in range(3):
                    nc.vector.bn_stats(out=st[:, sg], in_=sub[:, sg])
                mv = stat.tile([128, nc.vector.BN_AGGR_DIM], F32, tag="mv")
                nc.vector.bn_aggr(out=mv, in_=st)
                mean = mv[:, 0:1]
                rstd = stat.tile([128, 1], F32, tag="rstd")
                nc.scalar.activation(out=rstd, in_=mv[:, 1:2], func=AF.Sqrt,
                                     bias=eps[:, 0:1])
                nc.vector.reciprocal(out=rstd, in_=rstd)
                nc.vector.tensor_scalar(out=ut, in0=ut,
                                        scalar1=mean, scalar2=rstd,
                                        op0=OP.subtract, op1=OP.mult)
                pt = ps.tile([128, D_MODEL], F32, tag="big", bufs=2)
                for mt in range(NT_M):
                    nc.tensor.transpose(
                        out=pt[:, mt * 128:(mt + 1) * 128],
                        in_=ut[:, mt * 128:(mt + 1) * 128],
                        identity=ident,
                    )
                ptv = pt.rearrange("p (m f) -> p m f", m=NT_M)
                nc.scalar.activation(
                    out=unTv[:, :, lt * 128:(lt + 1) * 128],
                    in_=ptv, func=AF.Identity)

            # ---------- Phase 1: in_proj x -> conv -> silu, x_proj --------
            wx8 = bigw.tile([128, NT_M, D_INNER], FP8, tag="wx8")
            nc.sync.dma_start(out=wx8, in_=w_inx[:, :, :])
            wxp = []
            for kt in range(NT_DF):
                w = wsmall.tile([128, 128], BF16, tag=f"wxp{kt}")
                nc.sync.dma_start(out=w, in_=w_xproj[kt])
                wxp.append(w)

            xb_sb = wsmall.tile([128, NT_DF], F32, tag="xb")
            nc.sync.dma_start(out=xb_sb, in_=x_bias[:, :])
            cb_sb = wsmall.tile([128, NT_DF], F32, tag="cb")
            nc.sync.dma_start(out=cb_sb, in_=conv_b[:, :])

            zpad32 = wsmall.tile([128, K_CONV - 1], F32, tag="zpad32")
            nc.vector.memset(zpad32, 0.0)
            zpad = wsmall.tile([128, K_CONV - 1], FP8, tag="zpad")
            nc.vector.tensor_copy(out=zpad, in_=zpad32)

            pt_xp = ps.tile([128, L], F32, tag="xp", bufs=1)
            x_ssm = [None] * NT_DF
            for ddt in range(NT_DF):
                cin = scratch.tile([128, L + K_CONV - 1], FP8, tag="scrb",
                                   bufs=3)
                nc.vector.tensor_copy(out=cin[:, 0:K_CONV - 1], in_=zpad)
                pt = ps.tile([128, L], F32, tag="big", bufs=2)
                for lc in range(NLC):
                    for kp in range(NT_M // 2):
                        nc.tensor.matmul(
                            out=pt[:, lc * 512:(lc + 1) * 512],
                            lhsT=wx8[:, 2 * kp:2 * kp + 2,
                                     ddt * 128:(ddt + 1) * 128],
                            rhs=unTv[:, 2 * kp:2 * kp + 2,
                                     lc * 512:(lc + 1) * 512],
                            start=(kp == 0), stop=(kp == NT_M // 2 - 1),
                            perf_mode=DR,
                        )
                nc.scalar.activation(out=cin[:, K_CONV - 1:], in_=pt,
                                     func=AF.Identity, scale=1.0 / WXS,
                                     bias=xb_sb[:, ddt:ddt + 1])
                dg = wstream.tile([128, K_CONV, 128], FP8, tag="diag", bufs=2)
                nc.sync.dma_start(out=dg, in_=conv_diag[ddt])
                shard = NT_DS <= ddt
                if shard:
                    xs = xssmp.tile([128, L], BF16, tag="xssm", bufs=NT_DS)
                else:
                    xs = xssmp.tile([128, L], BF16, tag="xtmp", bufs=2)
                x_ssm[ddt] = xs
                ptc = ps.tile([128, L], F32, tag="conv", bufs=1)
                for lc in range(NLC):
                    for kp in range(K_CONV // 2):
                        base = cin[:, lc * 512 + 2 * kp: lc * 512 + 2 * kp + 1]
                        rhs = bass.AP(tensor=base.tensor, offset=base.offset,
                                      ap=[list(base.ap[0]), [1, 2], [1, 512]])
                        nc.tensor.matmul(
                            out=ptc[:, lc * 512:(lc + 1) * 512],
                            lhsT=dg[:, 2 * kp:2 * kp + 2],
                            rhs=rhs,
                            start=(kp == 0), stop=(kp == K_CONV // 2 - 1),
                            perf_mode=DR,
                        )
                nc.scalar.activation(out=xs, in_=ptc, func=AF.Silu,
                                     scale=1.0 / WCS,
                                     bias=cb_sb[:, ddt:ddt + 1])
                # interleaved x_proj accumulation (open psum group on pt_xp)
                for lc in range(NLC):
                    nc.tensor.matmul(
                        out=pt_xp[:, lc * 512:(lc + 1) * 512],
                        lhsT=wxp[ddt],
                        rhs=xs[:, lc * 512:(lc + 1) * 512],
                        start=(ddt == 0), stop=(ddt == NT_DF - 1),
                        skip_group_check=True,
                    )

            x_dbl = xdblp.tile([128, L], BF16, tag="xdbl")
            nc.scalar.activation(out=x_dbl, in_=pt_xp, func=AF.Identity)
            # park B/C rows (bf16) for partition-broadcast reload
            nc.sync.dma_start(out=bc_dram[:, :], in_=x_dbl[DT_RANK:128, :])

            # resident broadcast B/C tiles (4 rows per DMA)
            bt_all = bcp.tile([128, N_STATE, L], BF16, tag="bta")
            ct_all = bcp.tile([128, N_STATE, L], BF16, tag="cta")
            for half, dst in ((0, bt_all), (1, ct_all)):
                for ch in range(0, N_STATE, 4):
                    src = bc_dram[half * N_STATE + ch: half * N_STATE + ch + 4, :]
                    src = bass.AP(tensor=src.tensor, offset=src.offset,
                                  ap=[[0, 128]] + [list(d) for d in src.ap])
                    nc.sync.dma_start(out=dst[:, ch:ch + 4], in_=src)
            bts = [bt_all[:, n] for n in range(N_STATE)]
            cts = [ct_all[:, n] for n in range(N_STATE)]

            # ---------- Phase 3: dt = softplus(dt_raw @ dt_w.T + b) -------
            wdt = wsmall.tile([DT_RANK, DSH], BF16, tag="wdt")
            nc.sync.dma_start(out=wdt, in_=w_dt[:, :])
            dtb_sb = wsmall.tile([128, NT_DS], F32, tag="dtb")
            nc.sync.dma_start(out=dtb_sb, in_=dt_bias[:, :])
            a_sb = wsmall.tile([128, NT_DS, N_STATE], F32, tag="asc")
            nc.sync.dma_start(out=a_sb, in_=a_sc[:, :, :])
            zb_sb = wsmall.tile([128, NT_DS], F32, tag="zb")
            nc.sync.dma_start(out=zb_sb, in_=z_bias[:, :])
            dd_sb = []
            for dtile in range(NT_DS):
                dd = wstream.tile([128, 128], BF16, tag="ddiag", bufs=NT_DS)
                nc.sync.dma_start(out=dd, in_=d_diag[dtile])
                dd_sb.append(dd)
            wz8 = bigw.tile([128, NT_M, DSH], FP8, tag="wz8")
            nc.sync.dma_start(out=wz8, in_=w_inz[:, :, :])
            wo8 = bigw.tile([128, NT_DS, D_MODEL], FP8, tag="wo8")
            nc.sync.dma_start(out=wo8, in_=w_out[:, :, :])

            dts = []
            dtxs = []
            e16s = []
            for dtile in range(NT_DS):
                pt = ps.tile([128, L], F32, tag="conv", bufs=1)
                for lc in range(NLC):
                    nc.tensor.matmul(
                        out=pt[:, lc * 512:(lc + 1) * 512],
                        lhsT=wdt[:, dtile * 128:(dtile + 1) * 128],
                        rhs=x_dbl[0:DT_RANK, lc * 512:(lc + 1) * 512],
                        start=True, stop=True,
                    )
                # softplus = ln(1 + exp(v + b)); exp parked bf16 in rot rings
                etag = ["abar", "bu"][dtile % 2]
                e16 = rotp.tile([128, L], BF16, tag=etag, bufs=3,
                                name=f"e16_{dtile}")
                nc.scalar.activation(out=e16, in_=pt, func=AF.Exp,
                                     bias=dtb_sb[:, dtile:dtile + 1])
                e16s.append(e16)
            for dtile in range(NT_DS):
                dt_t = dtp.tile([128, L], BF16, tag="dt", bufs=NT_DS)
                nc.scalar.activation(out=dt_t, in_=e16s[dtile], func=AF.Ln,
                                     bias=1.0)
                dts.append(dt_t)
                dtx = dtp.tile([128, L], BF16, tag="dtx", bufs=NT_DS)
                nc.vector.tensor_tensor(out=dtx, in0=dt_t,
                                        in1=x_ssm[NT_DS + dtile], op=OP.mult)
                dtxs.append(dtx)

            # ---------- Phase 5: z matmuls (fp8 DR), scan loop ------------
            z_psum = []
            for zt in range(NT_DS):
                ptz = ps.tile([128, L], F32, tag=["xp", "conv"][zt % 2], bufs=1,
                              name=f"ptz{zt}")
                for lc in range(NLC):
                    for kp in range(NT_M // 2):
                        nc.tensor.matmul(
                            out=ptz[:, lc * 512:(lc + 1) * 512],
                            lhsT=wz8[:, 2 * kp:2 * kp + 2,
                                     zt * 128:(zt + 1) * 128],
                            rhs=unTv[:, 2 * kp:2 * kp + 2,
                                     lc * 512:(lc + 1) * 512],
                            start=(kp == 0), stop=(kp == NT_M // 2 - 1),
                            perf_mode=DR,
                            skip_group_check=True,
                        )
                z_psum.append(ptz)

            z_all = zgp.tile([128, NT_DS, L], BF16, tag="z")
            g_all = zgp.tile([128, NT_DS, L], FP8, tag="g")
            for dtile in range(NT_DS):
                y0ps = ps.tile([128, L], F32, tag="big", bufs=2)
                xsf = x_ssm[NT_DS + dtile]
                # seed y = D * x via diagonal matmul
                for lc in range(NLC):
                    nc.tensor.matmul(
                        out=y0ps[:, lc * 512:(lc + 1) * 512],
                        lhsT=dd_sb[dtile],
                        rhs=xsf[:, lc * 512:(lc + 1) * 512],
                        start=True, stop=False,
                        skip_group_check=True,
                    )
                for n in range(N_STATE):
                    abar = rotp.tile([128, L], BF16, tag="abar", bufs=3)
                    nc.scalar.activation(out=abar, in_=dts[dtile], func=AF.Exp,
                                         scale=a_sb[:, dtile, n:n + 1])
                    bu = rotp.tile([128, L], BF16, tag="bu", bufs=3)
                    bu_eng = nc.gpsimd if n in POOL_BU_N else nc.vector
                    bu_eng.tensor_tensor(out=bu, in0=dtxs[dtile], in1=bts[n],
                                         op=OP.mult)
                    h = rotp.tile([128, L], BF16, tag="h", bufs=3)
                    nc.vector.tensor_tensor_scan(out=h, data0=abar, data1=bu,
                                                 initial=0.0, op0=OP.mult,
                                                 op1=OP.add)
                    tmp = rotp.tile([128, L], BF16, tag="tmp", bufs=3)
                    tmp_eng = nc.gpsimd if n in POOL_TMP_N else nc.vector
                    tmp_eng.tensor_tensor(out=tmp, in0=h, in1=cts[n],
                                          op=OP.mult)
                    last = n == N_STATE - 1
                    for lc in range(NLC):
                        nc.tensor.matmul(
                            out=y0ps[:, lc * 512:(lc + 1) * 512],
                            lhsT=ident_bf,
                            rhs=tmp[:, lc * 512:(lc + 1) * 512],
                            start=False, stop=last,
                            skip_group_check=True,
                        )
                if dtile == 0:
                    for zt in range(NT_DS):
                        nc.scalar.activation(out=z_all[:, zt], in_=z_psum[zt],
                                             func=AF.Silu, scale=1.0 / WXS,
                                             bias=zb_sb[:, zt:zt + 1])
                # gate: g = (GS * y) * silu(z), fp8 for the out_proj rhs
                nc.vector.scalar_tensor_tensor(
                    out=g_all[:, dtile], in0=y0ps, scalar=GS,
                    in1=z_all[:, dtile], op0=OP.mult, op1=OP.mult)

            # ---------- Phase 6: out_proj (fp8 DR) ------------------------
            for mt in range(NT_M):
                otag, obufs = [("big", 2), ("conv", 1), ("xp", 1)][mt % 3]
                pt = ps.tile([128, L], F32, tag=otag, bufs=obufs)
                for lc in range(NLC):
                    for kp in range(NT_DS // 2):
                        nc.tensor.matmul(
                            out=pt[:, lc * 512:(lc + 1) * 512],
                            lhsT=wo8[:, 2 * kp:2 * kp + 2,
                                     mt * 128:(mt + 1) * 128],
                            rhs=g_all[:, 2 * kp:2 * kp + 2,
                                      lc * 512:(lc + 1) * 512],
                            start=(kp == 0), stop=(kp == NT_DS // 2 - 1),
                            perf_mode=DR,
                        )
                for lc in range(NLC):
                    ot = dtp.tile([128, 512], F32, tag="dtx", bufs=NT_DS,
                                  name="ot")
                    nc.scalar.activation(out=ot,
                                         in_=pt[:, lc * 512:(lc + 1) * 512],
                                         func=AF.Identity,
                                         scale=1.0 / (WOS * GS))
                    nc.sync.dma_start(out=outp[mt][:, lc * 512:(lc + 1) * 512],
                                      in_=ot)

    nc.finalize()
    return nc


def _bf16(a):
    return np.ascontiguousarray(np.asarray(a, np.float32)).astype(ml_dtypes.bfloat16)


def _fp8(a):
    return np.ascontiguousarray(np.asarray(a, np.float32)).astype(
        ml_dtypes.float8_e4m3)


def _shard_inputs(inputs):
    """Build the 8 per-core input maps. Core c: batch c>>2, branch (c>>1)&1,
    half c&1."""
    u = np.ascontiguousarray(np.asarray(inputs["u"], np.float32))
    norm_w = np.asarray(inputs["norm_w"], np.float32)
    norm_b = np.asarray(inputs["norm_b"], np.float32)

    in_maps = []
    meta = []
    for c in range(8):
        b, r, h = c >> 2, (c >> 1) & 1, c & 1
        pre = "fwd_" if r == 0 else "bwd_"
        in_w = np.asarray(inputs[pre + "in_w"], np.float32)
        conv_w = np.asarray(inputs[pre + "conv_w"], np.float32).reshape(D_INNER, K_CONV)
        conv_b = np.asarray(inputs[pre + "conv_b"], np.float32)
        A_log = np.asarray(inputs[pre + "A_log"], np.float32)
        xproj_w = np.asarray(inputs[pre + "xproj_w"], np.float32)
        dt_w = np.asarray(inputs[pre + "dt_w"], np.float32)
        dt_b = np.asarray(inputs[pre + "dt_b"], np.float32)
        D_p = np.asarray(inputs[pre + "D"], np.float32)
        out_w = np.asarray(inputs["out_w"], np.float32)

        sh = slice(h * DSH, (h + 1) * DSH)
        # channel order inside this core's program: non-shard half first,
        # the scanned shard last (the program scans x tiles 6..11)
        perm = np.r_[np.arange((1 - h) * DSH, (2 - h) * DSH),
                     np.arange(h * DSH, (h + 1) * DSH)]

        ub = u[b] if r == 0 else u[b, ::-1]
        u_t = np.ascontiguousarray(ub.reshape(NT_L, 128, D_MODEL))

        # fold norm affine into in_proj
        in_w_eff = in_w * norm_w[None, :]
        bias_full = in_w @ norm_b            # (2*D_INNER,)

        w_x = in_w_eff[:D_INNER][perm]       # (1536, 768), permuted
        w_z = in_w_eff[D_INNER:][sh]         # (768, 768)
        # [d_model, out] -> [128, NT_M, out] (partition=within-k-tile row)
        w_inx = _fp8((w_x.T * WXS).reshape(NT_M, 128, D_INNER).transpose(1, 0, 2))
        w_inz = _fp8((w_z.T * WXS).reshape(NT_M, 128, DSH).transpose(1, 0, 2))
        x_bias = np.ascontiguousarray(
            bias_full[:D_INNER][perm].reshape(NT_DF, 128).T)
        z_bias = np.ascontiguousarray(bias_full[D_INNER:][sh].reshape(NT_DS, 128).T)

        conv_w_p = conv_w[perm]
        cd = np.zeros((NT_DF, 128, K_CONV, 128), np.float32)
        idx = np.arange(128)
        for g in range(NT_DF):
            for k in range(K_CONV):
                cd[g, idx, k, idx] = conv_w_p[g * 128:(g + 1) * 128, k] * WCS
        cd = _fp8(cd)
        conv_bias = np.ascontiguousarray(conv_b[perm].reshape(NT_DF, 128).T)

        w_xp = _bf16(xproj_w[:, perm].T.reshape(NT_DF, 128, 128))
        w_dt_t = _bf16(dt_w[sh].T)              # (96, 768)
        dtb = np.ascontiguousarray(dt_b[sh].reshape(NT_DS, 128).T)
        A = -np.exp(A_log[sh].astype(np.float64)).astype(np.float32)   # (768, 16)
        a_sc = np.ascontiguousarray(A.reshape(NT_DS, 128, N_STATE).transpose(1, 0, 2))
        dd = np.zeros((NT_DS, 128, 128), np.float32)
        Dsh = D_p[sh]
        for g in range(NT_DS):
            dd[g, idx, idx] = Dsh[g * 128:(g + 1) * 128]
        dd = _bf16(dd)

        col = slice(r * D_INNER + h * DSH, r * D_INNER + (h + 1) * DSH)
        w_o = _fp8((out_w[:, col].T * WOS).reshape(NT_DS, 128, D_MODEL)
                   .transpose(1, 0, 2))

        in_maps.append({
            "u_in": u_t, "w_inx": w_inx, "w_inz": w_inz,
            "x_bias": x_bias, "z_bias": z_bias,
            "conv_diag": cd, "conv_b": conv_bias,
            "w_xproj": w_xp, "w_dt": w_dt_t, "dt_bias": dtb,
            "a_sc": a_sc, "d_diag": dd, "w_out": w_o,
        })
        meta.append((b, r, h))
    return in_maps, meta


def kernel(**inputs):
    global LAST_RESULTS
    nc = _build_program()
    in_maps, meta = _shard_inputs(inputs)
    trace = os.environ.get("KBENCH_TRACE", "0") == "1"
    res = run_bass_kernel_spmd(nc, in_maps, core_ids=list(range(8)), trace=trace)
    LAST_RESULTS = res

    u = np.asarray(inputs["u"], np.float32)
    out = np.array(u, np.float32, copy=True)
    for c, (b, r, h) in enumerate(meta):
        p = np.asarray(res.results[c]["outp"], np.float32).reshape(D_MODEL, L).T
        if r == 1:
            p = p[::-1]
        out[b] += p
    return out
